# revision 1
# baseline (speedup 1.0000x reference)
"""Trainium2 Bass kernel for BondUpdate GNN message passing.

Computes, for each bond e:
    x = concat(sites[idx1[e]], sites[idx2[e]], bonds[e], states[g2b[e]])  # [896]
    out[e] = relu(relu(relu(x @ W1 + b1) @ W2 + b2) @ W3 + b3)           # [256]

Strategy: shard bonds across 8 NeuronCores (data parallel over edges),
replicate sites/states tables and MLP weights. On each core, activations
live transposed in SBUF (features on partitions, bonds on free dim) so the
three matmul layers chain with no intermediate transposes. Row gathers use
indirect DMA; PE transposes move gathered rows into the transposed layout.
Matmul operands are bf16 (cast in-flight by the gather/weight DMAs; PSUM
accumulation stays fp32, biases+relu applied from PSUM in fp32, and the
output stage stays in 32-bit float32r so final values are not re-rounded).
Full fp32 streams at 4 cycles/column on the PE; bf16/f32r stream at 1.
Observed end-to-end error ~4e-3 relative to the fp32 reference scale.
"""
import sys

if "/opt/trn_rl_repo" not in sys.path:
    sys.path.insert(0, "/opt/trn_rl_repo")

import numpy as np

import concourse.bass as bass
import concourse.mybir as mybir
import concourse.tile as tile
from concourse.bass_utils import run_bass_kernel_spmd
from concourse.masks import make_identity
from concourse.vector_clock import ScopedClock

F32 = mybir.dt.float32
F32R = mybir.dt.float32r
BF16 = mybir.dt.bfloat16
I32 = mybir.dt.int32

P = 128            # partitions
T = 512            # bonds per tile
SUB = T // P       # 128-bond subtiles per tile

N_SITES = 20000
N_GRAPHS = 512
SITE_LEN = 256
BOND_LEN = 256
STATE_LEN = 128
IN_DIM = 2 * SITE_LEN + BOND_LEN + STATE_LEN  # 896
H1 = 1024
H2 = 1024
OUT_DIM = 256

KC1, MC1 = IN_DIM // P, H1 // P    # 7, 8
KC2, MC2 = H1 // P, H2 // P        # 8, 8
KC3, MC3 = H2 // P, OUT_DIM // P   # 8, 2

N_CORES = 8
N_BONDS = 200000
TILES_PER_CORE = 49                # 49*512 = 25088 bonds/core
E_CORE = TILES_PER_CORE * T        # 25088
E_PAD = N_CORES * E_CORE           # 200704


EVSEM_WAIT_CAP = 2  # InstEventSemaphore holds 2 waits; every other inst 1


def _legalize_waits(nc: bass.Bass):
    """Spill sync waits beyond the per-instruction capacity onto standalone
    InstEventSemaphore instructions inserted just before the offender.
    Walrus here rejects instructions with more waits than the ISA slots."""
    n_spilled = 0
    for f in nc.m.functions:
        for bb in f.blocks:
            il = bb.instructions
            i = 0
            while i < len(il):
                inst = il[i]
                si = inst.sync_info
                waits = list(si.on_wait) if si and si.on_wait else []
                cap = (
                    EVSEM_WAIT_CAP
                    if isinstance(inst, mybir.InstEventSemaphore)
                    else 1
                )
                if len(waits) > cap:
                    keep = waits[-cap:]
                    spill = waits[:-cap]
                    si.on_wait = keep
                    evs = []
                    for j in range(0, len(spill), EVSEM_WAIT_CAP):
                        ev = mybir.InstEventSemaphore(
                            name=nc.get_next_instruction_name(),
                            ins=[],
                            outs=[],
                            sync_info=mybir.SyncInfo(
                                on_wait=spill[j:j + EVSEM_WAIT_CAP],
                                on_update=[],
                            ),
                        )
                        ev.engine = inst.engine
                        nc.register_instruction(ev)
                        evs.append(ev)
                    il[i:i] = evs
                    i += len(evs)
                    n_spilled += len(spill)
                i += 1
    return n_spilled


class SplitDrainTileContext(tile.TileContext):
    """TileContext whose kernel-tail drain also respects the wait cap."""

    def _drain_and_barrier(self, tick_clock, wait_clock):
        nc = self.nc
        drain_inst = nc.sync.drain()
        wait_clock.add_sem_waits(
            drain_inst.ins, ScopedClock({None: tick_clock.global_clock})
        )
        si = drain_inst.ins.sync_info
        waits = list(si.on_wait or [])
        if len(waits) > 1:
            si.on_wait = []
            id2sem = {s.num: s for s in self.sems.allocated().values()}
            for w in waits:
                assert w.wait_mode == "sem-ge-imm", w
                nc.sync.wait_ge(id2sem[w.id], w.wait_value)
        nc.all_engine_barrier()
        assert self.sems is not None
        popped = nc._tile_sem_poison_stack.pop()
        assert popped is self._sem_poison
        nc.clear_and_free_semaphores(list(self.sems.allocated().values()))
        nc.all_engine_barrier()


def build_bass(n_tiles: int) -> bass.Bass:
    """Build the per-core Bass program processing n_tiles*T bonds."""
    nc = bass.Bass("TRN2", target_bir_lowering=False, debug=False, num_devices=1)
    E = n_tiles * T

    sites = nc.dram_tensor("sites", [N_SITES, SITE_LEN], F32, kind="ExternalInput")
    bondsT = nc.dram_tensor("bondsT", [BOND_LEN, E], F32, kind="ExternalInput")
    states = nc.dram_tensor("states", [N_GRAPHS, STATE_LEN], F32, kind="ExternalInput")
    # indices pre-wrapped on host to [P, n_tiles*SUB]: idx[p, t*SUB+j] = raw[t*T + j*P + p]
    idx1 = nc.dram_tensor("idx1", [P, n_tiles * SUB], I32, kind="ExternalInput")
    idx2 = nc.dram_tensor("idx2", [P, n_tiles * SUB], I32, kind="ExternalInput")
    g2b = nc.dram_tensor("g2b", [P, n_tiles * SUB], I32, kind="ExternalInput")
    # weights pre-chunked on host: w1c[p, (k*MC1+m)*P+j] = W1[k*P+p, m*P+j]
    w1c = nc.dram_tensor("w1c", [P, KC1 * MC1 * P], F32, kind="ExternalInput")
    w2c = nc.dram_tensor("w2c", [P, KC2 * MC2 * P], F32, kind="ExternalInput")
    w3c = nc.dram_tensor("w3c", [P, KC3 * MC3 * P], F32, kind="ExternalInput")
    # biases pre-wrapped: bXc[p, m] = bX[m*P+p]
    b1c = nc.dram_tensor("b1c", [P, MC1], F32, kind="ExternalInput")
    b2c = nc.dram_tensor("b2c", [P, MC2], F32, kind="ExternalInput")
    b3c = nc.dram_tensor("b3c", [P, MC3], F32, kind="ExternalInput")
    outT = nc.dram_tensor("outT", [OUT_DIM, E], F32, kind="ExternalOutput")


    with SplitDrainTileContext(nc) as tc:
        with (
            tc.tile_pool(name="const", bufs=1) as constp,
            tc.tile_pool(name="wts", bufs=1) as wp,
            tc.tile_pool(name="idx", bufs=1) as idxp,
            tc.tile_pool(name="stage", bufs=4) as stagep,
            tc.tile_pool(name="xT", bufs=3) as xp,
            tc.tile_pool(name="acts", bufs=1) as hp,
            tc.tile_pool(name="psx", bufs=4, space="PSUM") as psx,
            tc.tile_pool(name="psmm", bufs=4, space="PSUM") as psmm,
        ):
            ident_bf = constp.tile([P, P], BF16)
            make_identity(nc, ident_bf[:])

            b1sb = constp.tile([P, MC1], F32)
            b2sb = constp.tile([P, MC2], F32)
            b3sb = constp.tile([P, MC3], F32)
            nc.scalar.dma_start(b1sb[:], b1c[:, :])
            nc.scalar.dma_start(b2sb[:], b2c[:, :])
            nc.scalar.dma_start(b3sb[:], b3c[:, :])
            w1sb = wp.tile([P, KC1 * MC1 * P], BF16)
            w2sb = wp.tile([P, KC2 * MC2 * P], BF16)
            w3sb = wp.tile([P, KC3 * MC3 * P], BF16)
            nc.gpsimd.dma_start(w1sb[:], w1c[:, :])

            idx1sb = idxp.tile([P, n_tiles * SUB], I32)
            idx2sb = idxp.tile([P, n_tiles * SUB], I32)
            g2bsb = idxp.tile([P, n_tiles * SUB], I32)
            nc.sync.dma_start(idx1sb[:], idx1[:, :])
            nc.sync.dma_start(idx2sb[:], idx2[:, :])
            nc.sync.dma_start(g2bsb[:], g2b[:, :])

            for t in range(n_tiles):
                c0 = t * SUB
                # ---- gather rows: [P, SUB, row_len] (bond j*P+p -> [p, j, :])
                s1 = stagep.tile([P, SUB, SITE_LEN], BF16, tag="s1")
                s2 = stagep.tile([P, SUB, SITE_LEN], BF16, tag="s2")
                ss = stagep.tile([P, SUB, STATE_LEN], BF16, tag="ss")
                for j in range(SUB):
                    cj = c0 + j
                    nc.gpsimd.indirect_dma_start(
                        out=s1[:, j, :], out_offset=None, in_=sites[:],
                        in_offset=bass.IndirectOffsetOnAxis(
                            ap=idx1sb[:, cj:cj + 1], axis=0),
                    )
                    nc.gpsimd.indirect_dma_start(
                        out=s2[:, j, :], out_offset=None, in_=sites[:],
                        in_offset=bass.IndirectOffsetOnAxis(
                            ap=idx2sb[:, cj:cj + 1], axis=0),
                    )
                    nc.gpsimd.indirect_dma_start(
                        out=ss[:, j, :], out_offset=None, in_=states[:],
                        in_offset=bass.IndirectOffsetOnAxis(
                            ap=g2bsb[:, cj:cj + 1], axis=0),
                    )
                # bonds arrive pre-transposed from the host: cast-DMA chunks
                xb = []
                for c in range(2):
                    xsb = xp.tile([P, T], BF16, tag=f"xTb{c}")
                    nc.gpsimd.dma_start(
                        xsb[:], bondsT[c * P:(c + 1) * P, t * T:(t + 1) * T])
                    xb.append(xsb)

                # ---- transpose gathered rows into xT chunks [P, T]
                chunk_src = [(s1, 0), (s1, 1), (s2, 0), (s2, 1),
                             (None, 0), (None, 1), (ss, 0)]
                xT = []
                for k, (src, c) in enumerate(chunk_src):
                    if src is None:
                        xT.append(xb[c])
                        continue
                    pst = psx.tile([P, T], BF16, tag="psx")
                    for j in range(SUB):
                        nc.tensor.transpose(
                            pst[:, j * P:(j + 1) * P],
                            src[:, j, c * P:(c + 1) * P],
                            ident_bf[:],
                        )
                    xsb = xp.tile([P, T], BF16, tag=f"xT{k}")
                    nc.vector.tensor_copy(xsb[:], pst[:])
                    xT.append(xsb)

                # ---- layer 1: h1T[m] = relu(sum_k W1[k,m].T @ xT[k] + b1[m])
                h1T = []
                for m in range(MC1):
                    ps = psmm.tile([P, T], F32, tag="psmm")
                    for k in range(KC1):
                        nc.tensor.matmul(
                            ps[:],
                            w1sb[:, (k * MC1 + m) * P:(k * MC1 + m + 1) * P],
                            xT[k][:],
                            start=(k == 0), stop=(k == KC1 - 1),
                        )
                    hsb = hp.tile([P, T], BF16, tag=f"h1T{m}")
                    nc.scalar.activation(
                        hsb[:], ps[:], mybir.ActivationFunctionType.Relu,
                        bias=b1sb[:, m:m + 1],
                    )
                    h1T.append(hsb)

                if t == 0:
                    # deferred so tile 0's gathers lead the SWDGE queue
                    nc.gpsimd.dma_start(w2sb[:], w2c[:, :])
                    nc.gpsimd.dma_start(w3sb[:], w3c[:, :])

                # ---- layer 2
                h2T = []
                for m in range(MC2):
                    ps = psmm.tile([P, T], F32, tag="psmm")
                    for k in range(KC2):
                        nc.tensor.matmul(
                            ps[:],
                            w2sb[:, (k * MC2 + m) * P:(k * MC2 + m + 1) * P],
                            h1T[k][:],
                            start=(k == 0), stop=(k == KC2 - 1),
                        )
                    hsb = hp.tile([P, T], BF16, tag=f"h2T{m}")
                    nc.scalar.activation(
                        hsb[:], ps[:], mybir.ActivationFunctionType.Relu,
                        bias=b2sb[:, m:m + 1],
                    )
                    h2T.append(hsb)

                # ---- layer 3
                oT = []
                for m in range(MC3):
                    ps = psmm.tile([P, T], F32, tag="psmm")
                    for k in range(KC3):
                        nc.tensor.matmul(
                            ps[:],
                            w3sb[:, (k * MC3 + m) * P:(k * MC3 + m + 1) * P],
                            h2T[k][:],
                            start=(k == 0), stop=(k == KC3 - 1),
                        )
                    hsb = hp.tile([P, T], F32R, tag=f"oT{m}")
                    nc.scalar.activation(
                        hsb[:], ps[:], mybir.ActivationFunctionType.Relu,
                        bias=b3sb[:, m:m + 1],
                    )
                    oT.append(hsb)

                # ---- store transposed output; host un-transposes
                for c in range(MC3):
                    nc.sync.dma_start(
                        outT[c * P:(c + 1) * P, t * T:(t + 1) * T],
                        oT[c][:].bitcast(F32),
                    )

    _legalize_waits(nc)
    return nc


def _prep_shared(W1, b1, W2, b2, W3, b3):
    def chunk_w(W, KC, MC):
        # [KC*P, MC*P] -> [P, KC*MC*P] with w[p, (k*MC+m)*P+j] = W[k*P+p, m*P+j]
        return np.ascontiguousarray(
            W.reshape(KC, P, MC, P).transpose(1, 0, 2, 3).reshape(P, KC * MC * P)
        ).astype(np.float32, copy=False)

    def chunk_b(b, MC):
        return np.ascontiguousarray(b.reshape(MC, P).T).astype(np.float32, copy=False)

    return {
        "w1c": chunk_w(np.asarray(W1), KC1, MC1),
        "w2c": chunk_w(np.asarray(W2), KC2, MC2),
        "w3c": chunk_w(np.asarray(W3), KC3, MC3),
        "b1c": chunk_b(np.asarray(b1), MC1),
        "b2c": chunk_b(np.asarray(b2), MC2),
        "b3c": chunk_b(np.asarray(b3), MC3),
    }


def _wrap_idx(raw: np.ndarray) -> np.ndarray:
    # [E_core] -> [P, n_tiles*SUB] with idx[p, q] = raw[q*P + p]
    n = raw.shape[0] // P
    return np.ascontiguousarray(raw.reshape(n, P).T).astype(np.int32, copy=False)


_BUILT = {}


def _get_bass(n_tiles: int) -> bass.Bass:
    if n_tiles not in _BUILT:
        _BUILT[n_tiles] = build_bass(n_tiles)
    return _BUILT[n_tiles]


def make_in_maps(sites, bonds, states, indices1, indices2, graph_to_bonds,
                 W1, b1, W2, b2, W3, b3, n_cores=N_CORES,
                 tiles_per_core=TILES_PER_CORE):
    """Shard + reformat full inputs into per-core in_maps."""
    e_core = tiles_per_core * T
    e_pad = n_cores * e_core
    n_bonds = bonds.shape[0]

    sites = np.ascontiguousarray(np.asarray(sites), dtype=np.float32)
    states = np.ascontiguousarray(np.asarray(states), dtype=np.float32)

    bondsT_pad = np.zeros((BOND_LEN, e_pad), dtype=np.float32)
    bondsT_pad[:, :n_bonds] = np.asarray(bonds).T
    idx1_pad = np.zeros(e_pad, dtype=np.int32)
    idx1_pad[:n_bonds] = indices1
    idx2_pad = np.zeros(e_pad, dtype=np.int32)
    idx2_pad[:n_bonds] = indices2
    g2b_pad = np.zeros(e_pad, dtype=np.int32)
    g2b_pad[:n_bonds] = graph_to_bonds

    shared = _prep_shared(W1, b1, W2, b2, W3, b3)
    in_maps = []
    for c in range(n_cores):
        lo, hi = c * e_core, (c + 1) * e_core
        m = {
            "sites": sites,
            "states": states,
            "bondsT": np.ascontiguousarray(bondsT_pad[:, lo:hi]),
            "idx1": _wrap_idx(idx1_pad[lo:hi]),
            "idx2": _wrap_idx(idx2_pad[lo:hi]),
            "g2b": _wrap_idx(g2b_pad[lo:hi]),
        }
        m.update(shared)
        in_maps.append(m)
    return in_maps


def kernel(sites, bonds, states, indices1, indices2, graph_to_bonds,
           W1, b1, W2, b2, W3, b3):
    n_bonds = bonds.shape[0]
    in_maps = make_in_maps(sites, bonds, states, indices1, indices2,
                           graph_to_bonds, W1, b1, W2, b2, W3, b3)
    nc = _get_bass(TILES_PER_CORE)
    res = run_bass_kernel_spmd(nc, in_maps, core_ids=list(range(N_CORES)))
    out_t = np.concatenate([res.results[c]["outT"] for c in range(N_CORES)], axis=1)
    return np.ascontiguousarray(out_t.T[:n_bonds])

